# revision 37
# baseline (speedup 1.0000x reference)
"""BitNet linear layer (b1.58-style) on 8 Trainium2 NeuronCores.

Computes: scale = 1e-4 + mean(|W|); q = clip(round(W/scale), -1, 1);
          out = scale * (x @ q.T)
for x [4, 2048, 2048] f32 and W [8192, 2048] f32.

Sharding: tensor-parallel over out_features. Each core gets the full x
(replicated) and a 1024-row shard of the ternary q; cores run fully
independently and the host concatenates the per-core [8192, 1024]
output slices along the feature axis.

The elementwise prep runs once on the host (it is ~0.1% of the FLOPs
and would otherwise be redundantly recomputed per core): the exact
global scale and ternary q (bit-identical rounding vs the reference),
the narrowing casts, and the transposes into SBUF-ready layouts. The
scale is applied on-device during the psum drain (a [128,1] f32 input)
because the two precision halves below must share one accumulator.

Mixed-precision contraction — the core idea. The bf16 matmul issue
floor is 216 ns per N=512 matmul (1 moving col/cycle at 2.4 GHz), i.e.
443 us for the full 2048-deep contraction. fp8 DoubleRow mode packs 2
weights per PE cell and streams 2 fp8 moving elements/cycle: measured
216 ns for a contraction-256 N=512 matmul — a genuine 2x. Full fp8 is
too lossy for the 2e-2 gate (e4m3 on x alone gives ~2.5% out error),
but a *split* contraction works: k-tiles 0-7 run bf16 x (exact-ish),
k-tiles 8-15 run fp8 x in DoubleRow pairs, both accumulating into the
same psum bank. Measured rel err on the reference inputs: 1.70e-2
(15% inside the gate, deterministic); stream time per m-tile drops
from 32x216 to 24x216 ns = 25% fewer PE cycles.

DoubleRow semantics (validated on HW, exact): lhsT [p, i, m] fp8,
rhs [p, i, n] fp8 (i = 0,1 the packed pair), out[m, n] +=
sum_p sum_i lhsT[p,i,m] * rhs[p,i,n]; the host lays out both operands
with the same (p, i) -> k mapping: k = 1024 + kp*256 + i*128 + p.

Startup/steady schedule (bounded by per-queue DMA rates, the ~12-13 us
first-completion floor every queue shows regardless of size, and the
shared HBM/SBUF-write fabric):
  - The critical startup set is split across three queues to race
    three first-completion floors in parallel: q slices on gpsimd
    (SWDGE, ~2x faster than HWDGE for these strided 128-partition
    tiles), x0/x1 bf16 chunks on scalar, late q slices on sync.
  - m-tiles 0 and 1 run ko-major interleaved so each q slice is
    consumed twice per arrival; steady-state pairs likewise (and one
    ~50 ns semaphore hiccup per pair instead of two).
  - ~13 dummy matmuls on a zeroed SBUF tile (output never read) carry
    the PE through the HAM SHORT window up to the data floor so real
    matmuls start at 2.4 GHz (idle default is 1.2 GHz).
  - Steady-state x rides gpsimd as two-m-tile pair DMAs (bf16 and fp8
    parts), emitted after the burst in program order — a natural
    throttle that keeps prefetch from starving the startup burst.
  - Per m-tile: 8 bf16 k-steps + 4 DoubleRow k-pair-steps of two
    512-col matmuls into a psum bank pair; DVE drains psum * scale ->
    bf16 out tile; stores on sync in natural [M, N-shard] orientation.
    The final m-tile runs sweeps of shrinking width (512/256/256) so
    the last serial chain is one 256-col drain + 64 KiB store.
"""

import os
import sys

sys.path.insert(0, "/opt/trn_rl_repo")
os.environ.setdefault("JAX_PLATFORMS", "axon")

import numpy as np
import ml_dtypes

import concourse.bass as bass
import concourse.tile as tile
from concourse import bacc, mybir
from concourse.bass_utils import run_bass_kernel_spmd

F32 = mybir.dt.float32
BF16 = mybir.dt.bfloat16
FP8 = mybir.dt.float8e4
U32 = mybir.dt.uint32
BF16_NP = ml_dtypes.bfloat16
FP8_NP = ml_dtypes.float8_e4m3
DR = mybir.MatmulPerfMode.DoubleRow

NCORES = 8
M = 8192          # tokens (4*2048)
K = 2048          # in_features
N_FULL = 8192     # out_features
NS = N_FULL // NCORES  # 1024 per-core shard
P = 128
KB = 6            # bf16 k-tiles (k < 768)
KP = 5            # fp8 DoubleRow k-pairs (k >= 768, 256 each)
MT = M // P       # 64 m-tiles
NPAIR = MT // 2
XBW = KB * P      # bf16 x cols per m-tile (768)
X8W = KP * 2 * P  # fp8 x cols per m-tile (1280)


def build_nc():
    nc = bacc.Bacc("TRN2", target_bir_lowering=False, debug=False,
                   num_devices=NCORES)
    # xb rows pair*128+p: [j, ko*128+m] -> x[(2*pair+j)*128+m, ko*128+p]
    xb_d = nc.dram_tensor("xb", [M // 2, 2 * XBW], BF16, kind="ExternalInput")
    # x8 rows pair*128+p: [j, kp, i, m] -> x[(2*pair+j)*128+m, 1024+kp*256+i*128+p]
    x8_d = nc.dram_tensor("x8", [M // 2, 2 * X8W], FP8, kind="ExternalInput")
    # qa[p, ko*1024+n] = q[n, ko*128+p]            (bf16-half moving operand)
    qa_d = nc.dram_tensor("qa", [P, KB * NS], FP8, kind="ExternalInput")
    # qd[p, (kp, i, n)] = q[n, 1024+kp*256+i*128+p] (DoubleRow moving operand)
    qd_d = nc.dram_tensor("qd", [P, KP * 2 * NS], FP8, kind="ExternalInput")
    sc_d = nc.dram_tensor("sc", [P, 1], F32, kind="ExternalInput")
    o_d = nc.dram_tensor("out", [M, NS], BF16, kind="ExternalOutput")
    xb_ap, x8_ap = xb_d.ap(), x8_d.ap()
    qa_ap, qd_ap, sc_ap, o_ap = qa_d.ap(), qd_d.ap(), sc_d.ap(), o_d.ap()

    with tile.TileContext(nc) as tc:
        with (
            tc.tile_pool(name="qpool", bufs=1) as qpool,
            tc.tile_pool(name="xspool", bufs=4) as xspool,
            tc.tile_pool(name="x8spool", bufs=4) as x8spool,
            tc.tile_pool(name="xbpool", bufs=4) as xbpool,
            tc.tile_pool(name="x8pool", bufs=4) as x8pool,
            tc.tile_pool(name="opool", bufs=4) as opool,
            tc.tile_pool(name="psum_o", bufs=8, space="PSUM") as psum_o,
        ):
            tile_qa = qpool.tile([P, KB * NS], FP8, name="qa")
            tile_qd = qpool.tile([P, KP, 2, NS], FP8, name="qd")
            sc = qpool.tile([P, 1], F32, name="sc")
            warm = qpool.tile([P, 640], BF16, name="warm")

            # ---- PE warmup (HAM) --------------------------------------
            wz = warm[:].bitcast(U32)
            nc.vector.tensor_scalar(wz, wz, 0, None,
                                    mybir.AluOpType.bitwise_and)
            psW = psum_o.tile([P, 512], F32, name="psW", tag="ps")
            for _ in range(13):
                nc.tensor.matmul(psW[:], lhsT=warm[:, 0:P],
                                 rhs=warm[:, P:640], start=True, stop=True)

            # ---- startup burst ----------------------------------------
            nc.sync.dma_start(sc[:], sc_ap[:, :])
            # late-needed q: DoubleRow half split sync (slow but early
            # slices) / gpsimd-after-qa (fast, for the last slices the
            # sync queue would deliver too late)
            for g in range(KP - 2):
                nc.sync.dma_start(tile_qd[:, g, :, :],
                                  qd_ap[:, g * 2 * NS:(g + 1) * 2 * NS])

            # x0/x1 bf16 chunks on the otherwise-idle scalar queue
            xt0 = xspool.tile([P, XBW], BF16, name="xb_0", tag="x")
            xt1 = xspool.tile([P, XBW], BF16, name="xb_1", tag="x")
            def x01_chunk(xt, mt, c0, c1):
                nc.scalar.dma_start(
                    xt[:, c0:c1],
                    xb_ap[0:P, mt * XBW + c0:mt * XBW + c1])
            for c0, c1 in ((0, 512), (512, XBW)):
                x01_chunk(xt0, 0, c0, c1); x01_chunk(xt1, 1, c0, c1)
            # fp8 x halves for mt0/1 (needed from k-step 8, ~7us in)
            x80 = x8spool.tile([P, 1, KP, 2, P], FP8, name="x8_0", tag="x8")
            x81 = x8spool.tile([P, 1, KP, 2, P], FP8, name="x8_1", tag="x8")
            nc.scalar.dma_start(x80[:], x8_ap[0:P, 0:X8W])
            nc.scalar.dma_start(x81[:], x8_ap[0:P, X8W:2 * X8W])

            # critical bf16-half q on gpsimd, first slice in 512-granules
            nc.gpsimd.dma_start(tile_qa[:, 0:512], qa_ap[:, 0:512])
            nc.gpsimd.dma_start(tile_qa[:, 512:1024], qa_ap[:, 512:1024])
            nc.gpsimd.dma_start(tile_qa[:, 1024:2048], qa_ap[:, 1024:2048])
            for g in range(1, KB // 2):
                nc.gpsimd.dma_start(
                    tile_qa[:, 2 * g * NS:2 * (g + 1) * NS],
                    qa_ap[:, 2 * g * NS:2 * (g + 1) * NS])
            for g in range(KP - 2, KP):
                nc.gpsimd.dma_start(tile_qd[:, g, :, :],
                                    qd_ap[:, g * 2 * NS:(g + 1) * 2 * NS])

            # x singles for m-tiles 2-3
            def xs_single(mt):
                xt = xspool.tile([P, XBW], BF16, name=f"xb_{mt}", tag="x")
                nc.gpsimd.dma_start(
                    xt[:], xb_ap[(mt // 2) * P:(mt // 2 + 1) * P,
                                 (mt % 2) * XBW:(mt % 2 + 1) * XBW])
                x8t = x8spool.tile([P, 1, KP, 2, P], FP8, name=f"x8_{mt}",
                                   tag="x8")
                nc.gpsimd.dma_start(
                    x8t[:], x8_ap[(mt // 2) * P:(mt // 2 + 1) * P,
                                  (mt % 2) * X8W:(mt % 2 + 1) * X8W])
                return xt, x8t
            xt2, x82 = xs_single(2)
            xt3, x83 = xs_single(3)

            # ---- main loop: out[m, n] = sum_k x[m,k] q[n,k] -----------
            def mm_bf16(ps2, xt, base, ko, start, stop):
                lhsT = xt[:, base + ko * P:base + (ko + 1) * P]
                nc.tensor.matmul(
                    ps2[0][:], lhsT=lhsT,
                    rhs=tile_qa[:, ko * NS:ko * NS + 512],
                    start=start, stop=stop)
                nc.tensor.matmul(
                    ps2[1][:], lhsT=lhsT,
                    rhs=tile_qa[:, ko * NS + 512:(ko + 1) * NS],
                    start=start, stop=stop)

            def mm_dr(ps2, x8t, j8, kp, start, stop):
                lhsT = x8t[:, j8, kp, :, :]
                nc.tensor.matmul(
                    ps2[0][:], lhsT=lhsT, rhs=tile_qd[:, kp, :, 0:512],
                    start=start, stop=stop, perf_mode=DR)
                nc.tensor.matmul(
                    ps2[1][:], lhsT=lhsT, rhs=tile_qd[:, kp, :, 512:NS],
                    start=start, stop=stop, perf_mode=DR)

            def drain_store(mt, ps2):
                ot = opool.tile([P, NS], BF16, name=f"o_{mt}", tag="o")
                nc.vector.tensor_scalar(
                    ot[:, 0:512], ps2[0][:], sc[:], None,
                    mybir.AluOpType.mult)
                nc.vector.tensor_scalar(
                    ot[:, 512:1024], ps2[1][:], sc[:], None,
                    mybir.AluOpType.mult)
                nc.sync.dma_start(o_ap[mt * P:(mt + 1) * P, :], ot[:])

            def ps_pair(mt):
                return (psum_o.tile([P, 512], F32, name=f"psA_{mt}", tag="ps"),
                        psum_o.tile([P, 512], F32, name=f"psB_{mt}", tag="ps"))

            # note: alternating dr_first per pair (to halve the ~190ns
            # bf16<->DoubleRow mode transitions) measured 2.3us SLOWER
            # than always bf16-first, reproducibly — keep bf16-first
            def sweep_interleaved(mts, xts, x8ts, bases, bases8, dr_first):
                pss = [ps_pair(mt) for mt in mts]
                def bf16_block(start_mode):
                    for ko in range(KB):
                        for ps2, xt, b in zip(pss, xts, bases):
                            mm_bf16(ps2, xt, b, ko,
                                    start_mode and ko == 0,
                                    (not start_mode) and ko == KB - 1)
                def dr_block(start_mode):
                    for kp in range(KP):
                        for ps2, x8t, b8 in zip(pss, x8ts, bases8):
                            mm_dr(ps2, x8t, b8, kp,
                                  start_mode and kp == 0,
                                  (not start_mode) and kp == KP - 1)
                if dr_first:
                    dr_block(True)
                    bf16_block(False)
                else:
                    bf16_block(True)
                    dr_block(False)
                for mt, ps2 in zip(mts, pss):
                    drain_store(mt, ps2)

            # m-tiles 0/1 interleaved (startup), then 2/3
            sweep_interleaved((0, 1), (xt0, xt1), (x80, x81), (0, 0),
                              (0, 0), dr_first=False)
            sweep_interleaved((2, 3), (xt2, xt3), (x82, x83), (0, 0),
                              (0, 0), dr_first=False)


            # steady state: two-m-tile pair DMAs on gpsimd, processed
            # as 2-pair super-sweeps [bf16 A, bf16 B, DR A, DR B] —
            # same-mode blocks chain across pairs, halving the ~190ns
            # bf16<->DoubleRow transitions (2 per 4 m-tiles instead of
            # 2 per 2) while keeping bf16-first for every psum group.
            # Pair A's drains overlap pair B's DR block, so the full
            # 8-bank psum rotation has no WAR bubble.
            def pair_dma(pair):
                xbt = xbpool.tile([P, 2 * XBW], BF16, name=f"xbp_{pair}",
                                  tag="xbp")
                nc.gpsimd.dma_start(
                    xbt[:], xb_ap[pair * P:(pair + 1) * P, :])
                x8t = x8pool.tile([P, 2, KP, 2, P], FP8, name=f"x8p_{pair}",
                                  tag="x8p")
                nc.gpsimd.dma_start(
                    x8t[:], x8_ap[pair * P:(pair + 1) * P, :])
                return xbt, x8t

            def bf16_block(pss, xbt):
                for ko in range(KB):
                    for ps2, b in zip(pss, (0, XBW)):
                        mm_bf16(ps2, xbt, b, ko, ko == 0, False)

            def dr_block(pss, x8t):
                for kp in range(KP):
                    for ps2, j8 in zip(pss, (0, 1)):
                        mm_dr(ps2, x8t, j8, kp, False, kp == KP - 1)

            for s in range(2, NPAIR - 2, 2):
                xba, x8a = pair_dma(s)
                xbb, x8b = pair_dma(s + 1)
                ps_a = [ps_pair(2 * s), ps_pair(2 * s + 1)]
                ps_b = [ps_pair(2 * s + 2), ps_pair(2 * s + 3)]
                bf16_block(ps_a, xba)
                bf16_block(ps_b, xbb)
                dr_block(ps_a, x8a)
                dr_block(ps_b, x8b)
                for mt, ps2 in zip((2 * s, 2 * s + 1), ps_a):
                    drain_store(mt, ps2)
                for mt, ps2 in zip((2 * s + 2, 2 * s + 3), ps_b):
                    drain_store(mt, ps2)

            # pair NPAIR-2 as a single sweep (odd pair count before last)
            pair = NPAIR - 2
            xbt, x8t = pair_dma(pair)
            sweep_interleaved(
                (2 * pair, 2 * pair + 1), (xbt, xbt), (x8t, x8t),
                (0, XBW), (0, 1), dr_first=False)

            # last pair: sequential m-tiles; final m-tile in shrinking
            # widths so the last serial chain is a 256-col drain + 64 KiB
            pair = NPAIR - 1
            xbt = xbpool.tile([P, 2 * XBW], BF16, name=f"xbp_{pair}",
                              tag="xbp")
            nc.gpsimd.dma_start(xbt[:], xb_ap[pair * P:(pair + 1) * P, :])
            x8t = x8pool.tile([P, 2, KP, 2, P], FP8, name=f"x8p_{pair}",
                              tag="x8p")
            nc.gpsimd.dma_start(x8t[:], x8_ap[pair * P:(pair + 1) * P, :])

            mt = MT - 2
            ps = ps_pair(mt)
            for ko in range(KB):
                mm_bf16(ps, xbt, 0, ko, ko == 0, False)
            for kp in range(KP):
                mm_dr(ps, x8t, 0, kp, False, kp == KP - 1)
            drain_store(mt, ps)

            mt = MT - 1
            ot = opool.tile([P, NS], BF16, name=f"o_{mt}", tag="o")
            for n0, nw in ((0, 512), (512, 256), (768, 256)):
                ps1 = psum_o.tile([P, 512], F32, name=f"ps_{mt}_{n0}",
                                  tag="ps")
                for ko in range(KB):
                    nc.tensor.matmul(
                        ps1[:, 0:nw],
                        lhsT=xbt[:, XBW + ko * P:XBW + (ko + 1) * P],
                        rhs=tile_qa[:, ko * NS + n0:ko * NS + n0 + nw],
                        start=(ko == 0), stop=False)
                for kp in range(KP):
                    lhsT8 = x8t[:, 1, kp, :, :]
                    nc.tensor.matmul(
                        ps1[:, 0:nw], lhsT=lhsT8,
                        rhs=tile_qd[:, kp, :, n0:n0 + nw],
                        start=False, stop=(kp == KP - 1), perf_mode=DR)
                nc.vector.tensor_scalar(
                    ot[:, n0:n0 + nw], ps1[:, 0:nw], sc[:], None,
                    mybir.AluOpType.mult)
                nc.sync.dma_start(
                    o_ap[mt * P:(mt + 1) * P, n0:n0 + nw], ot[:, n0:n0 + nw])

    nc.compile()
    return nc


_NC_CACHE = None


def get_nc():
    global _NC_CACHE
    if _NC_CACHE is None:
        _NC_CACHE = build_nc()
    return _NC_CACHE


def make_in_maps(x, weight):
    x2 = np.asarray(x, dtype=np.float32).reshape(M, K)
    w = np.asarray(weight, dtype=np.float32)

    # exact reference prep: scale from the full W, ternary q
    scale = np.float32(1e-4) + np.abs(w).mean(dtype=np.float32)
    q = np.clip(np.rint(w / scale), -1.0, 1.0).astype(np.float32)

    # bf16 half: xb[pair*128+p, j*1024 + ko*128+m] = x[(2p+j)*128+m, ko*128+p]
    xlo = x2[:, :XBW].reshape(NPAIR, 2, P, KB, P)   # [pair, j, m, ko, p]
    xb = np.ascontiguousarray(
        xlo.transpose(0, 4, 1, 3, 2).reshape(M // 2, 2 * XBW).astype(BF16_NP))

    # fp8 half: x8[pair*128+p, j, kp, i, m] = x[(2p+j)*128+m, 1024+kp*256+i*128+p]
    xhi = x2[:, XBW:].reshape(NPAIR, 2, P, KP, 2, P)  # [pair, j, m, kp, i, p]
    x8 = np.ascontiguousarray(
        xhi.transpose(0, 5, 1, 3, 4, 2).reshape(M // 2, 2 * X8W).astype(FP8_NP))

    # qa[c, p, ko*1024+n] = q[c*1024+n, ko*128+p]
    qlo = q[:, :XBW].reshape(NCORES, NS, KB, P).transpose(0, 3, 2, 1)
    qa = np.ascontiguousarray(
        qlo.reshape(NCORES, P, KB * NS).astype(FP8_NP))

    # qd[c, p, kp, i, n] = q[c*1024+n, 1024+kp*256+i*128+p]
    qhi = q[:, XBW:].reshape(NCORES, NS, KP, 2, P).transpose(0, 4, 2, 3, 1)
    qd = np.ascontiguousarray(
        qhi.reshape(NCORES, P, KP * 2 * NS).astype(FP8_NP))

    sc = np.full((P, 1), scale, dtype=np.float32)
    return [{"xb": xb, "x8": x8, "qa": qa[c], "qd": qd[c], "sc": sc}
            for c in range(NCORES)]


def kernel(x, weight):
    nc = get_nc()
    in_maps = make_in_maps(x, weight)
    try:
        res = run_bass_kernel_spmd(nc, in_maps, list(range(NCORES)))
    except Exception:
        # transient device errors have been observed on first touch; retry once
        res = run_bass_kernel_spmd(nc, in_maps, list(range(NCORES)))
    out = np.concatenate(
        [np.asarray(res.results[c]["out"]) for c in range(NCORES)], axis=1)
    return np.ascontiguousarray(out, dtype=np.float32).reshape(4, 2048, N_FULL)


# revision 38
# speedup vs baseline: 1.0030x; 1.0030x over previous
"""BitNet linear layer (b1.58-style) on 8 Trainium2 NeuronCores.

Computes: scale = 1e-4 + mean(|W|); q = clip(round(W/scale), -1, 1);
          out = scale * (x @ q.T)
for x [4, 2048, 2048] f32 and W [8192, 2048] f32.

Sharding: tensor-parallel over out_features. Each core gets the full x
(replicated) and a 1024-row shard of the ternary q; cores run fully
independently and the host concatenates the per-core [8192, 1024]
output slices along the feature axis.

The elementwise prep runs once on the host (it is ~0.1% of the FLOPs
and would otherwise be redundantly recomputed per core): the exact
global scale and ternary q (bit-identical rounding vs the reference),
the narrowing casts, and the transposes into SBUF-ready layouts. The
scale is applied on-device during the psum drain (a [128,1] f32 input)
because the two precision halves below must share one accumulator.

Mixed-precision contraction — the core idea. The bf16 matmul issue
floor is 216 ns per N=512 matmul (1 moving col/cycle at 2.4 GHz), i.e.
443 us for the full 2048-deep contraction. fp8 DoubleRow mode packs 2
weights per PE cell and streams 2 fp8 moving elements/cycle: measured
216 ns for a contraction-256 N=512 matmul — a genuine 2x. Full fp8 is
too lossy for the 2e-2 gate (e4m3 on x alone gives ~2.5% out error),
but a *split* contraction works: k-tiles 0-7 run bf16 x (exact-ish),
k-tiles 8-15 run fp8 x in DoubleRow pairs, both accumulating into the
same psum bank. Measured rel err on the reference inputs: 1.70e-2
(15% inside the gate, deterministic); stream time per m-tile drops
from 32x216 to 24x216 ns = 25% fewer PE cycles.

DoubleRow semantics (validated on HW, exact): lhsT [p, i, m] fp8,
rhs [p, i, n] fp8 (i = 0,1 the packed pair), out[m, n] +=
sum_p sum_i lhsT[p,i,m] * rhs[p,i,n]; the host lays out both operands
with the same (p, i) -> k mapping: k = 1024 + kp*256 + i*128 + p.

Startup/steady schedule (bounded by per-queue DMA rates, the ~12-13 us
first-completion floor every queue shows regardless of size, and the
shared HBM/SBUF-write fabric):
  - The critical startup set is split across three queues to race
    three first-completion floors in parallel: q slices on gpsimd
    (SWDGE, ~2x faster than HWDGE for these strided 128-partition
    tiles), x0/x1 bf16 chunks on scalar, late q slices on sync.
  - m-tiles 0 and 1 run ko-major interleaved so each q slice is
    consumed twice per arrival; steady-state pairs likewise (and one
    ~50 ns semaphore hiccup per pair instead of two).
  - ~13 dummy matmuls on a zeroed SBUF tile (output never read) carry
    the PE through the HAM SHORT window up to the data floor so real
    matmuls start at 2.4 GHz (idle default is 1.2 GHz).
  - Steady-state x rides gpsimd as two-m-tile pair DMAs (bf16 and fp8
    parts), emitted after the burst in program order — a natural
    throttle that keeps prefetch from starving the startup burst.
  - Per m-tile: 8 bf16 k-steps + 4 DoubleRow k-pair-steps of two
    512-col matmuls into a psum bank pair; DVE drains psum * scale ->
    bf16 out tile; stores on sync in natural [M, N-shard] orientation.
    The final m-tile runs sweeps of shrinking width (512/256/256) so
    the last serial chain is one 256-col drain + 64 KiB store.
"""

import os
import sys

sys.path.insert(0, "/opt/trn_rl_repo")
os.environ.setdefault("JAX_PLATFORMS", "axon")

import numpy as np
import ml_dtypes

import concourse.bass as bass
import concourse.tile as tile
from concourse import bacc, mybir
from concourse.bass_utils import run_bass_kernel_spmd

F32 = mybir.dt.float32
BF16 = mybir.dt.bfloat16
FP8 = mybir.dt.float8e4
U32 = mybir.dt.uint32
BF16_NP = ml_dtypes.bfloat16
FP8_NP = ml_dtypes.float8_e4m3
DR = mybir.MatmulPerfMode.DoubleRow

NCORES = 8
M = 8192          # tokens (4*2048)
K = 2048          # in_features
N_FULL = 8192     # out_features
NS = N_FULL // NCORES  # 1024 per-core shard
P = 128
KB = 6            # bf16 k-tiles (k < 768)
KP = 5            # fp8 DoubleRow k-pairs (k >= 768, 256 each)
MT = M // P       # 64 m-tiles
NPAIR = MT // 2
XBW = KB * P      # bf16 x cols per m-tile (768)
X8W = KP * 2 * P  # fp8 x cols per m-tile (1280)


def build_nc():
    nc = bacc.Bacc("TRN2", target_bir_lowering=False, debug=False,
                   num_devices=NCORES)
    # xb rows pair*128+p: [j, ko*128+m] -> x[(2*pair+j)*128+m, ko*128+p]
    xb_d = nc.dram_tensor("xb", [M // 2, 2 * XBW], BF16, kind="ExternalInput")
    # x8 rows pair*128+p: [j, kp, i, m] -> x[(2*pair+j)*128+m, 1024+kp*256+i*128+p]
    x8_d = nc.dram_tensor("x8", [M // 2, 2 * X8W], FP8, kind="ExternalInput")
    # qa[p, ko*1024+n] = q[n, ko*128+p]            (bf16-half moving operand)
    qa_d = nc.dram_tensor("qa", [P, KB * NS], FP8, kind="ExternalInput")
    # qd[p, (kp, i, n)] = q[n, 1024+kp*256+i*128+p] (DoubleRow moving operand)
    qd_d = nc.dram_tensor("qd", [P, KP * 2 * NS], FP8, kind="ExternalInput")
    sc_d = nc.dram_tensor("sc", [P, 1], F32, kind="ExternalInput")
    o_d = nc.dram_tensor("out", [M, NS], BF16, kind="ExternalOutput")
    xb_ap, x8_ap = xb_d.ap(), x8_d.ap()
    qa_ap, qd_ap, sc_ap, o_ap = qa_d.ap(), qd_d.ap(), sc_d.ap(), o_d.ap()

    with tile.TileContext(nc) as tc:
        with (
            tc.tile_pool(name="qpool", bufs=1) as qpool,
            tc.tile_pool(name="xspool", bufs=4) as xspool,
            tc.tile_pool(name="x8spool", bufs=4) as x8spool,
            tc.tile_pool(name="xbpool", bufs=4) as xbpool,
            tc.tile_pool(name="x8pool", bufs=4) as x8pool,
            tc.tile_pool(name="opool", bufs=4) as opool,
            tc.tile_pool(name="psum_o", bufs=8, space="PSUM") as psum_o,
        ):
            tile_qa = qpool.tile([P, KB * NS], FP8, name="qa")
            tile_qd = qpool.tile([P, KP, 2, NS], FP8, name="qd")
            sc = qpool.tile([P, 1], F32, name="sc")
            warm = qpool.tile([P, 640], BF16, name="warm")

            # ---- PE warmup (HAM) --------------------------------------
            wz = warm[:].bitcast(U32)
            nc.vector.tensor_scalar(wz, wz, 0, None,
                                    mybir.AluOpType.bitwise_and)
            psW = psum_o.tile([P, 512], F32, name="psW", tag="ps")
            for _ in range(13):
                nc.tensor.matmul(psW[:], lhsT=warm[:, 0:P],
                                 rhs=warm[:, P:640], start=True, stop=True)

            # ---- startup burst ----------------------------------------
            nc.sync.dma_start(sc[:], sc_ap[:, :])
            # late-needed q: DoubleRow half split sync (slow but early
            # slices) / gpsimd-after-qa (fast, for the last slices the
            # sync queue would deliver too late)
            for g in range(KP - 2):
                nc.sync.dma_start(tile_qd[:, g, :, :],
                                  qd_ap[:, g * 2 * NS:(g + 1) * 2 * NS])

            # x0/x1 bf16 chunks on the otherwise-idle scalar queue
            xt0 = xspool.tile([P, XBW], BF16, name="xb_0", tag="x")
            xt1 = xspool.tile([P, XBW], BF16, name="xb_1", tag="x")
            def x01_chunk(xt, mt, c0, c1):
                nc.scalar.dma_start(
                    xt[:, c0:c1],
                    xb_ap[0:P, mt * XBW + c0:mt * XBW + c1])
            for c0, c1 in ((0, 512), (512, XBW)):
                x01_chunk(xt0, 0, c0, c1); x01_chunk(xt1, 1, c0, c1)
            # fp8 x halves for mt0/1 (needed from k-step 8, ~7us in)
            x80 = x8spool.tile([P, 1, KP, 2, P], FP8, name="x8_0", tag="x8")
            x81 = x8spool.tile([P, 1, KP, 2, P], FP8, name="x8_1", tag="x8")
            nc.scalar.dma_start(x80[:], x8_ap[0:P, 0:X8W])
            nc.scalar.dma_start(x81[:], x8_ap[0:P, X8W:2 * X8W])

            # critical bf16-half q on gpsimd, first slice in 512-granules
            nc.gpsimd.dma_start(tile_qa[:, 0:512], qa_ap[:, 0:512])
            nc.gpsimd.dma_start(tile_qa[:, 512:1024], qa_ap[:, 512:1024])
            nc.gpsimd.dma_start(tile_qa[:, 1024:2048], qa_ap[:, 1024:2048])
            for g in range(1, KB // 2):
                nc.gpsimd.dma_start(
                    tile_qa[:, 2 * g * NS:2 * (g + 1) * NS],
                    qa_ap[:, 2 * g * NS:2 * (g + 1) * NS])
            for g in range(KP - 2, KP):
                nc.gpsimd.dma_start(tile_qd[:, g, :, :],
                                    qd_ap[:, g * 2 * NS:(g + 1) * 2 * NS])

            # x singles for m-tiles 2-3
            def xs_single(mt):
                xt = xspool.tile([P, XBW], BF16, name=f"xb_{mt}", tag="x")
                nc.gpsimd.dma_start(
                    xt[:], xb_ap[(mt // 2) * P:(mt // 2 + 1) * P,
                                 (mt % 2) * XBW:(mt % 2 + 1) * XBW])
                x8t = x8spool.tile([P, 1, KP, 2, P], FP8, name=f"x8_{mt}",
                                   tag="x8")
                nc.gpsimd.dma_start(
                    x8t[:], x8_ap[(mt // 2) * P:(mt // 2 + 1) * P,
                                  (mt % 2) * X8W:(mt % 2 + 1) * X8W])
                return xt, x8t
            xt2, x82 = xs_single(2)
            xt3, x83 = xs_single(3)

            # ---- main loop: out[m, n] = sum_k x[m,k] q[n,k] -----------
            def mm_bf16(ps2, xt, base, ko, start, stop):
                lhsT = xt[:, base + ko * P:base + (ko + 1) * P]
                nc.tensor.matmul(
                    ps2[0][:], lhsT=lhsT,
                    rhs=tile_qa[:, ko * NS:ko * NS + 512],
                    start=start, stop=stop)
                nc.tensor.matmul(
                    ps2[1][:], lhsT=lhsT,
                    rhs=tile_qa[:, ko * NS + 512:(ko + 1) * NS],
                    start=start, stop=stop)

            def mm_dr(ps2, x8t, j8, kp, start, stop):
                lhsT = x8t[:, j8, kp, :, :]
                nc.tensor.matmul(
                    ps2[0][:], lhsT=lhsT, rhs=tile_qd[:, kp, :, 0:512],
                    start=start, stop=stop, perf_mode=DR)
                nc.tensor.matmul(
                    ps2[1][:], lhsT=lhsT, rhs=tile_qd[:, kp, :, 512:NS],
                    start=start, stop=stop, perf_mode=DR)

            def drain_store(mt, ps2):
                ot = opool.tile([P, NS], BF16, name=f"o_{mt}", tag="o")
                nc.vector.tensor_scalar(
                    ot[:, 0:512], ps2[0][:], sc[:], None,
                    mybir.AluOpType.mult)
                nc.vector.tensor_scalar(
                    ot[:, 512:1024], ps2[1][:], sc[:], None,
                    mybir.AluOpType.mult)
                nc.sync.dma_start(o_ap[mt * P:(mt + 1) * P, :], ot[:])

            def ps_pair(mt):
                return (psum_o.tile([P, 512], F32, name=f"psA_{mt}", tag="ps"),
                        psum_o.tile([P, 512], F32, name=f"psB_{mt}", tag="ps"))

            # note: alternating dr_first per pair (to halve the ~190ns
            # bf16<->DoubleRow mode transitions) measured 2.3us SLOWER
            # than always bf16-first, reproducibly — keep bf16-first
            def sweep_interleaved(mts, xts, x8ts, bases, bases8, dr_first):
                pss = [ps_pair(mt) for mt in mts]
                def bf16_block(start_mode):
                    for ko in range(KB):
                        for ps2, xt, b in zip(pss, xts, bases):
                            mm_bf16(ps2, xt, b, ko,
                                    start_mode and ko == 0,
                                    (not start_mode) and ko == KB - 1)
                def dr_block(start_mode):
                    for kp in range(KP):
                        for ps2, x8t, b8 in zip(pss, x8ts, bases8):
                            mm_dr(ps2, x8t, b8, kp,
                                  start_mode and kp == 0,
                                  (not start_mode) and kp == KP - 1)
                if dr_first:
                    dr_block(True)
                    bf16_block(False)
                else:
                    bf16_block(True)
                    dr_block(False)
                for mt, ps2 in zip(mts, pss):
                    drain_store(mt, ps2)

            # m-tiles 0/1 interleaved (startup), then 2/3
            sweep_interleaved((0, 1), (xt0, xt1), (x80, x81), (0, 0),
                              (0, 0), dr_first=False)
            sweep_interleaved((2, 3), (xt2, xt3), (x82, x83), (0, 0),
                              (0, 0), dr_first=False)


            # steady state: two-m-tile pair DMAs on gpsimd, processed
            # as 2-pair super-sweeps [bf16 A, bf16 B, DR A, DR B] —
            # same-mode blocks chain across pairs, halving the ~190ns
            # bf16<->DoubleRow transitions (2 per 4 m-tiles instead of
            # 2 per 2) while keeping bf16-first for every psum group.
            # Pair A's drains overlap pair B's DR block, so the full
            # 8-bank psum rotation has no WAR bubble.
            def pair_dma(pair):
                xbt = xbpool.tile([P, 2 * XBW], BF16, name=f"xbp_{pair}",
                                  tag="xbp")
                nc.gpsimd.dma_start(
                    xbt[:], xb_ap[pair * P:(pair + 1) * P, :])
                x8t = x8pool.tile([P, 2, KP, 2, P], FP8, name=f"x8p_{pair}",
                                  tag="x8p")
                nc.gpsimd.dma_start(
                    x8t[:], x8_ap[pair * P:(pair + 1) * P, :])
                return xbt, x8t

            def bf16_block(pss, xbt):
                for ko in range(KB):
                    for ps2, b in zip(pss, (0, XBW)):
                        mm_bf16(ps2, xbt, b, ko, ko == 0, False)

            def dr_block(pss, x8t):
                for kp in range(KP):
                    for ps2, j8 in zip(pss, (0, 1)):
                        mm_dr(ps2, x8t, j8, kp, False, kp == KP - 1)

            for s in range(2, NPAIR - 2, 2):
                xba, x8a = pair_dma(s)
                xbb, x8b = pair_dma(s + 1)
                ps_a = [ps_pair(2 * s), ps_pair(2 * s + 1)]
                ps_b = [ps_pair(2 * s + 2), ps_pair(2 * s + 3)]
                bf16_block(ps_a, xba)
                bf16_block(ps_b, xbb)
                dr_block(ps_a, x8a)
                dr_block(ps_b, x8b)
                for mt, ps2 in zip((2 * s, 2 * s + 1), ps_a):
                    drain_store(mt, ps2)
                for mt, ps2 in zip((2 * s + 2, 2 * s + 3), ps_b):
                    drain_store(mt, ps2)

            # pair NPAIR-2 as a single sweep (odd pair count before last)
            pair = NPAIR - 2
            xbt, x8t = pair_dma(pair)
            sweep_interleaved(
                (2 * pair, 2 * pair + 1), (xbt, xbt), (x8t, x8t),
                (0, XBW), (0, 1), dr_first=False)

            # last pair: sequential m-tiles; final m-tile in shrinking
            # widths so the last serial chain is a 256-col drain + 64 KiB
            pair = NPAIR - 1
            xbt = xbpool.tile([P, 2 * XBW], BF16, name=f"xbp_{pair}",
                              tag="xbp")
            nc.gpsimd.dma_start(xbt[:], xb_ap[pair * P:(pair + 1) * P, :])
            x8t = x8pool.tile([P, 2, KP, 2, P], FP8, name=f"x8p_{pair}",
                              tag="x8p")
            nc.gpsimd.dma_start(x8t[:], x8_ap[pair * P:(pair + 1) * P, :])

            mt = MT - 2
            ps = ps_pair(mt)
            for ko in range(KB):
                mm_bf16(ps, xbt, 0, ko, ko == 0, False)
            for kp in range(KP):
                mm_dr(ps, x8t, 0, kp, False, kp == KP - 1)
            drain_store(mt, ps)

            mt = MT - 1
            ot = opool.tile([P, NS], BF16, name=f"o_{mt}", tag="o")
            # last two stores go to different queues so their HBM write
            # receipts (~2us each, size-independent) overlap
            st_eng = {768: nc.scalar}
            for n0, nw in ((0, 512), (512, 256), (768, 128), (896, 128)):
                ps1 = psum_o.tile([P, 512], F32, name=f"ps_{mt}_{n0}",
                                  tag="ps")
                for ko in range(KB):
                    nc.tensor.matmul(
                        ps1[:, 0:nw],
                        lhsT=xbt[:, XBW + ko * P:XBW + (ko + 1) * P],
                        rhs=tile_qa[:, ko * NS + n0:ko * NS + n0 + nw],
                        start=(ko == 0), stop=False)
                for kp in range(KP):
                    lhsT8 = x8t[:, 1, kp, :, :]
                    nc.tensor.matmul(
                        ps1[:, 0:nw], lhsT=lhsT8,
                        rhs=tile_qd[:, kp, :, n0:n0 + nw],
                        start=False, stop=(kp == KP - 1), perf_mode=DR)
                nc.vector.tensor_scalar(
                    ot[:, n0:n0 + nw], ps1[:, 0:nw], sc[:], None,
                    mybir.AluOpType.mult)
                st_eng.get(n0, nc.sync).dma_start(
                    o_ap[mt * P:(mt + 1) * P, n0:n0 + nw], ot[:, n0:n0 + nw])

    nc.compile()
    return nc


_NC_CACHE = None


def get_nc():
    global _NC_CACHE
    if _NC_CACHE is None:
        _NC_CACHE = build_nc()
    return _NC_CACHE


def make_in_maps(x, weight):
    x2 = np.asarray(x, dtype=np.float32).reshape(M, K)
    w = np.asarray(weight, dtype=np.float32)

    # exact reference prep: scale from the full W, ternary q
    scale = np.float32(1e-4) + np.abs(w).mean(dtype=np.float32)
    q = np.clip(np.rint(w / scale), -1.0, 1.0).astype(np.float32)

    # bf16 half: xb[pair*128+p, j*1024 + ko*128+m] = x[(2p+j)*128+m, ko*128+p]
    xlo = x2[:, :XBW].reshape(NPAIR, 2, P, KB, P)   # [pair, j, m, ko, p]
    xb = np.ascontiguousarray(
        xlo.transpose(0, 4, 1, 3, 2).reshape(M // 2, 2 * XBW).astype(BF16_NP))

    # fp8 half: x8[pair*128+p, j, kp, i, m] = x[(2p+j)*128+m, 1024+kp*256+i*128+p]
    xhi = x2[:, XBW:].reshape(NPAIR, 2, P, KP, 2, P)  # [pair, j, m, kp, i, p]
    x8 = np.ascontiguousarray(
        xhi.transpose(0, 5, 1, 3, 4, 2).reshape(M // 2, 2 * X8W).astype(FP8_NP))

    # qa[c, p, ko*1024+n] = q[c*1024+n, ko*128+p]
    qlo = q[:, :XBW].reshape(NCORES, NS, KB, P).transpose(0, 3, 2, 1)
    qa = np.ascontiguousarray(
        qlo.reshape(NCORES, P, KB * NS).astype(FP8_NP))

    # qd[c, p, kp, i, n] = q[c*1024+n, 1024+kp*256+i*128+p]
    qhi = q[:, XBW:].reshape(NCORES, NS, KP, 2, P).transpose(0, 4, 2, 3, 1)
    qd = np.ascontiguousarray(
        qhi.reshape(NCORES, P, KP * 2 * NS).astype(FP8_NP))

    sc = np.full((P, 1), scale, dtype=np.float32)
    return [{"xb": xb, "x8": x8, "qa": qa[c], "qd": qd[c], "sc": sc}
            for c in range(NCORES)]


def kernel(x, weight):
    nc = get_nc()
    in_maps = make_in_maps(x, weight)
    try:
        res = run_bass_kernel_spmd(nc, in_maps, list(range(NCORES)))
    except Exception:
        # transient device errors have been observed on first touch; retry once
        res = run_bass_kernel_spmd(nc, in_maps, list(range(NCORES)))
    out = np.concatenate(
        [np.asarray(res.results[c]["out"]) for c in range(NCORES)], axis=1)
    return np.ascontiguousarray(out, dtype=np.float32).reshape(4, 2048, N_FULL)
